# revision 57
# baseline (speedup 1.0000x reference)
"""Multi-head Hyena FFT long conv with fused gating — TRN2 Bass kernel.

Problem: nn_MultiHeadHyenaConv (B=2, D=1024, L=2048, num_heads=8, H=128
filter channels).  Reference semantics:
    kv[b,h,d1,d2,l] = v[b,h,d1,l] * x2[b,h,d2,l]
    y = causal_conv_l(kv, k[h]) + bias[h,d1] * kv
    out[b,h,d2,l]   = sum_d1 y[b,h,d1,d2,l] * x1[b,h,d1,l]

Strategy (per the sharding hint): tensor-parallel over the H=128 head
channels -> 8 cores x 16 heads, both batches per core, zero cross-core
communication.  Inputs are sliced host-side, outputs concatenated.

Per-core kernel (_build2): the causal conv is a lower-triangular
Toeplitz matmul on the TensorEngine in fp16 (1 col/cycle) with fp32
PSUM accumulation:
  - KS[p, x] = k[x - p] shifted kernel images, one strided DMA per head
    from a host-side row-replicated staging array; KS[:, 128j:+128] is
    the stationary for diagonal offset j (16 per head, prefetched on
    both HWDGE rings).
  - Inputs arrive HOST-pre-transposed in [lam, (b, lc, d)] fp16 layout
    (plain 1MB DMAs; on-device xbar DMA-transpose measured ~10GB/s
    effective and cost ~70us of exposed prologue).
  - KV[m, (b,d1,d2)] forms on the otherwise-idle Pool engine (gpsimd).
  - Conv matmuls coalesced per psum bank (<=512 moving cols — the ISA
    s3d3_mm_num_elements cap forbids spanning banks), 40 matmuls/head,
    psum double-buffered across heads.
  - DVE gates straight from PSUM with a d2-broadcast x1 AP (no X1R
    replication, no ACT drain), then an in-place d1-reduction tree.
  - Bias skip term x2 * W (W = sum_d1 bias*x1*v, 2 fused DVE ops in the
    prologue) is added on Pool; outputs are stored head-major and DMA
    out incrementally per head; the host detangles the layout.

DMA ring discipline: the two HWDGE rings are in-order queues, so ALL
loads (inputs + KS) go on the SP ring and ALL output stores on the ACT
ring — with mixed assignment, a next-iteration load descriptor queued
behind the final head's store serializes ~5-8us at every invocation
boundary (measured 152.4k vs 161.1k ns same-process).

Measured (8 cores, axon, interleaved min-timing — sequential timing of
the two builds carries cluster-drift bias): rel err 8.5e-4; full-kernel
steady state ~153.7us/invocation (hardware rep-loop wrapping input DMA
+ W + 16-head loop + out DMA, ping-ponged input tiles; reproduced to
+-70ns).  The PE matmul stream IS the floor: a zero-sem-wait probe of
the bare 640 LDW+matmul stream runs ~155us vs the 116us column
roofline (per-instruction overhead ~134cy — consistent with no cross-
instruction array pipelining; LDW count irrelevant — dedup of
consecutive identical loads is active in the compile hook; instruction
count pinned at 40/head by the 512-col psum bank cap).  DVE/Pool/ACT/
DMA all hide behind it.  Rejected by measurement: ACT drain16 gating
(+47us), KV on DVE (+37us), pipeline depth 2 (+20us), tail finals on
DVE (+18us), grouped KS DMAs (+19-26us: fewer PE waits but each gates
on a 2MB transfer), full-span matmuls (ISA reject), fp8 (error
budget), on-device transposes (slow xbar).
"""
import os
import sys

import numpy as np

for _p in ("/opt/trn_rl_repo", "/root/.axon_site/_ro/trn_rl_repo"):
    if os.path.isdir(_p) and _p not in sys.path:
        sys.path.insert(0, _p)

import concourse.bass as bass
import concourse.mybir as mybir
from concourse.bass_utils import run_bass_kernel_spmd
from concourse.tile import TileContext

f16, f32 = mybir.dt.float16, mybir.dt.float32

# --- workaround: this walrus build accepts at most ONE sem wait per
# instruction (TPB_EVENTS has a single wait slot) and refuses to split
# them itself for DMA/CTRL opcodes.  Post-process the BIR json: move
# extra waits onto single-wait NoOps inserted just before, same engine.
import json as _json

import concourse.bass_utils as _bu
import concourse.bass2jax as _b2j


def _split_multiwaits(bir_json):
    j = _json.loads(bir_json)
    changed = False
    for fn in j.get("functions", []):
        for bb in fn.get("blocks", []):
            insts = bb.get("instructions", [])
            out, ctr = [], 0
            for ins in insts:
                si = ins.get("sync_info")
                waits = (si or {}).get("on_wait") or []
                if len(waits) > 1:
                    changed = True
                    for w in waits[:-1]:
                        out.append({
                            "name": f"{ins['name']}-wsplit{ctr}",
                            "opcode": "NoOp",
                            "engine": ins["engine"],
                            "ins": [], "outs": [],
                            "debug": ins.get("debug", 0),
                            "sync_info": {"on_wait": [w], "on_update": []},
                        })
                        ctr += 1
                    si["on_wait"] = [waits[-1]]
                out.append(ins)
            if ctr:
                bb["instructions"] = out
    if not changed:
        return bir_json
    return _json.dumps(j).encode()




# --- LDWEIGHTS dedup: walrus (ldw-opt is broken/disabled) re-emits an
# identical LDWEIGHTS before every matmul.  Consecutive byte-identical
# LDW records (same weights AP, same idempotent sem wait) reload weights
# the PE array already holds — overwrite them with NOPs in the packed
# NEFF (same 64B record size, no address shifts).
import io as _io
import tarfile as _tarfile
import tempfile as _tempfile

from concourse import neff as _neff


def _dedup_ldweights_in_neff(neff_path):
    with open(neff_path, "rb") as f:
        old_header = f.read(1024)
        tar_data = f.read()
    with _tempfile.TemporaryDirectory() as d:
        with _tarfile.open(fileobj=_io.BytesIO(tar_data), mode="r") as t:
            t.extractall(d)
        pe_path = os.path.join(d, "sg00", "PE0.bin")
        if not os.path.exists(pe_path):
            return
        data = bytearray(open(pe_path, "rb").read())
        nop = bytes([0xA4, 0x10, 0, 0]) + bytes(60)
        last_key, n = None, 0
        for i in range(0, len(data), 64):
            rec = bytes(data[i:i + 64])
            if rec[0] == 0x01:  # LDWEIGHTS
                key = rec[:2] + rec[4:]
                if key == last_key:
                    data[i:i + 64] = nop
                    n += 1
                else:
                    last_key = key
            elif rec[0] != 0xA4:  # any non-NOP invalidates tracking
                if rec[0] != 0x02:  # MATMUL keeps array weights
                    last_key = None
        if not n:
            return
        open(pe_path, "wb").write(bytes(data))
        buf = _io.BytesIO()
        with _tarfile.open(fileobj=buf, mode="w") as t:
            t.add(d, arcname=".", filter=_b2j._reset_tarinfo)
        new_data = buf.getvalue()
        new_header = _neff.make_deterministic_neff_header(
            old_neff_header=old_header, new_neff_data=new_data)
    with open(neff_path, "wb") as f:
        f.write(new_header + new_data)


_orig_compile_bir_kernel = _bu.compile_bir_kernel


def _compile_bir_kernel_splitwaits(bir_json, tmpdir, neff_name="file.neff"):
    neff_path = _orig_compile_bir_kernel(_split_multiwaits(bir_json), tmpdir, neff_name)
    try:
        _dedup_ldweights_in_neff(neff_path)
    except Exception as e:  # non-fatal: unpatched NEFF is still correct
        print(f"ldw-dedup skipped: {e!r}")
    return neff_path


if _bu.compile_bir_kernel is not _compile_bir_kernel_splitwaits:
    _bu.compile_bir_kernel = _compile_bir_kernel_splitwaits
    _b2j.compile_bir_kernel = _compile_bir_kernel_splitwaits

MUL, ADD = mybir.AluOpType.mult, mybir.AluOpType.add

B, D, L, NH = 2, 1024, 2048, 8
N_CORES = 8
H = D // NH              # 128 filter channels
HL = H // N_CORES        # 16 heads per core
DL = HL * NH             # 128 data channels per core
NT = L // 128            # 16 sequence tiles
KP = L + 127             # padded kernel row length


def _build(nc: bass.Bass, heads: int = HL, conv: bool = True, gate: bool = True, reps: int = 1, coalesce: bool = True, direct_gate: bool = False, dma_tr: bool = True, ks_prefetch: bool = True):
    v16 = nc.dram_tensor("v16", [B, DL, L], f16, kind="ExternalInput")
    x1_16 = nc.dram_tensor("x1_16", [B, DL, L], f16, kind="ExternalInput")
    x2_16 = nc.dram_tensor("x2_16", [B, DL, L], f16, kind="ExternalInput")
    x1b16 = nc.dram_tensor("x1b16", [B, DL, L], f16, kind="ExternalInput")
    kstag16 = nc.dram_tensor("kstag16", [HL, 128, KP], f16, kind="ExternalInput")
    ident_h = nc.dram_tensor("ident_h", [DL, DL], f16, kind="ExternalInput")
    identb_h = nc.dram_tensor("identb_h", [DL, DL], f16, kind="ExternalInput")
    ident_s = nc.dram_tensor("ident_s", [128, 128], f32, kind="ExternalInput")
    out_d = nc.dram_tensor("out", [B, DL, L], f32, kind="ExternalOutput")

    with TileContext(nc) as tc:
        with tc.tile_pool(name="const", bufs=1) as constp, \
             tc.tile_pool(name="persist", bufs=1) as persist:
            idh = constp.tile([DL, DL], f16)
            idbh = constp.tile([DL, DL], f16)
            ids = constp.tile([128, 128], f32)
            nc.sync.dma_start(out=idh[:], in_=ident_h[:])
            nc.sync.dma_start(out=idbh[:], in_=identb_h[:])
            nc.sync.dma_start(out=ids[:], in_=ident_s[:])

            # persistent transposed inputs: [128 lam, (b, lc, d)] fp16
            vT = persist.tile([128, B * NT * DL], f16)
            x1T = persist.tile([128, B * NT * DL], f16)
            x1bT = persist.tile([128, B * NT * DL], f16)
            x2T = persist.tile([128, B * NT * DL], f16)
            WT = persist.tile([128, B * NT * HL], f32)
            outT32 = persist.tile([128, B * NT * DL], f32)
            out_nat = persist.tile([DL, B * L], f32)

            # ---- transposed input loads ----
            if dma_tr:
                # xbar DMA transpose straight from HBM; x1*bias precomputed host-side
                for b in range(B):
                    for (src, dstT) in ((v16, vT), (x2_16, x2T), (x1_16, x1T), (x1b16, x1bT)):
                        for lc in range(NT):
                            col = (b * NT + lc) * DL
                            nc.sync.dma_start_transpose(
                                out=dstT[:, col:col + DL],
                                in_=src[b][:, lc * 128:(lc + 1) * 128])
            else:
              with tc.tile_pool(name="nat", bufs=3) as natp, \
                 tc.tile_pool(name="pst", bufs=3, space="PSUM") as pst:
                for b in range(B):
                    for (src, dstT) in ((v16, vT), (x2_16, x2T), (x1_16, x1T)):
                        nat = natp.tile([DL, L], f16, tag="nat")
                        nc.sync.dma_start(out=nat[:], in_=src[b])
                        for lc in range(NT):
                            ps = pst.tile([128, DL], f16, tag="ps")
                            nc.tensor.transpose(ps[:], nat[:, lc * 128:(lc + 1) * 128], idh[:])
                            col = (b * NT + lc) * DL
                            nc.scalar.copy(out=dstT[:, col:col + DL], in_=ps[:])
                            if dstT is x1T:  # bias-scaled transpose via plain matmul
                                ps2 = pst.tile([128, DL], f32, tag="ps2")
                                nc.tensor.matmul(ps2[:], nat[:, lc * 128:(lc + 1) * 128], idbh[:],
                                                 start=True, stop=True)
                                nc.scalar.copy(out=x1bT[:, col:col + DL], in_=ps2[:])

            # ---- W term: WT[lam, (b, lc, h)] = sum_d1 bias*x1*v ----
            with tc.tile_pool(name="wtmp", bufs=1) as wtmp:
                t = wtmp.tile([128, B * NT * DL], f16)
                nc.vector.tensor_tensor(out=t[:], in0=x1bT[:], in1=vT[:], op=MUL)
                n_grp = B * NT * HL
                for width in (4, 2):
                    a0 = bass.AP(t[:].tensor, t[:].offset, [[B * NT * DL, 128], [NH, n_grp], [1, width]])
                    a1 = bass.AP(t[:].tensor, t[:].offset + width, [[B * NT * DL, 128], [NH, n_grp], [1, width]])
                    nc.vector.tensor_tensor(out=a0, in0=a0, in1=a1, op=ADD)
                a0 = bass.AP(t[:].tensor, t[:].offset, [[B * NT * DL, 128], [NH, n_grp]])
                a1 = bass.AP(t[:].tensor, t[:].offset + 1, [[B * NT * DL, 128], [NH, n_grp]])
                nc.vector.tensor_tensor(out=WT[:], in0=a0, in1=a1, op=ADD)

            # ---- main per-head loop ----
            with tc.tile_pool(name="ks", bufs=HL) as ksp, \
                 tc.tile_pool(name="kv", bufs=3) as kvp, \
                 tc.tile_pool(name="x1r", bufs=3) as x1rp, \
                 tc.tile_pool(name="y16", bufs=3) as y16p, \
                 tc.tile_pool(name="g", bufs=3) as gp, \
                 tc.tile_pool(name="psc", bufs=2, space="PSUM") as psc:
              if True:
                import contextlib
                rep_ctx = tc.For_i(0, reps, 1) if reps > 1 else contextlib.nullcontext()
                with rep_ctx:
                 if True:
                    front = {}
                    # prefetch all KS images up front: the 512KB strided DMA
                    # (~5us each) otherwise serializes per head; alternate the
                    # two HWDGE rings (SP / ACT) for parallel drain
                    KS_all = {}
                    if ks_prefetch:
                        for h in range(heads):
                            KS = ksp.tile([128, L], f16, tag="ks")
                            ksrc = bass.AP(kstag16[:].tensor, h * 128 * KP + 127,
                                           [[KP - 1, 128], [1, L]])
                            eng = nc.sync if h % 2 == 0 else nc.scalar
                            eng.dma_start(out=KS[:], in_=ksrc)
                            KS_all[h] = KS

                    def emit_front(h):
                        if ks_prefetch:
                            KS = KS_all[h]
                        else:
                            KS = ksp.tile([128, L], f16, tag="ks")
                            ksrc = bass.AP(kstag16[:].tensor, h * 128 * KP + 127,
                                           [[KP - 1, 128], [1, L]])
                            nc.sync.dma_start(out=KS[:], in_=ksrc)
                        KV = kvp.tile([128, NT * 128], f16, tag="kv")
                        for b in range(B):
                            i0 = bass.AP(vT[:].tensor, vT[:].offset + b * NT * DL + h * NH,
                                         [[B * NT * DL, 128], [DL, NT], [1, NH], [0, NH]])
                            i1 = bass.AP(x2T[:].tensor, x2T[:].offset + b * NT * DL + h * NH,
                                         [[B * NT * DL, 128], [DL, NT], [0, NH], [1, NH]])
                            o = bass.AP(KV[:].tensor, KV[:].offset + b * NH * NH,
                                        [[NT * 128, 128], [128, NT], [NH, NH], [1, NH]])
                            nc.vector.tensor_tensor(out=o, in0=i0, in1=i1, op=MUL)
                        X1R = x1rp.tile([128, NT * 128], f16, tag="x1r")
                        for b in range(B):
                            i0 = bass.AP(x1T[:].tensor, x1T[:].offset + b * NT * DL + h * NH,
                                         [[B * NT * DL, 128], [DL, NT], [1, NH], [0, NH]])
                            o = bass.AP(X1R[:].tensor, X1R[:].offset + b * NH * NH,
                                        [[NT * 128, 128], [128, NT], [NH, NH], [1, NH]])
                            nc.scalar.copy(out=o, in_=i0)
                        front[h] = (KS, KV, X1R)

                    def emit_conv(h):
                        KS, KV, X1R = front[h]
                        psum = psc.tile([128, NT * 128], f32, tag="ps")
                        NJ = NT if conv is True else int(conv)
                        for j in range(NJ):
                            lhsT = KS[:, j * 128:(j + 1) * 128]
                            if coalesce:
                                for bk in range(NT // 4):
                                    lo = max(j, 4 * bk)
                                    hi = 4 * bk + 3
                                    if lo > hi:
                                        continue
                                    cnt = hi - lo + 1
                                    mi0 = lo - j
                                    nc.tensor.matmul(
                                        psum[:, lo * 128:(hi + 1) * 128], lhsT,
                                        KV[:, mi0 * 128:(mi0 + cnt) * 128],
                                        start=(j == 0), stop=(j == min(hi, NJ - 1)))
                            else:
                                for li in range(j, NT):
                                    mi = li - j
                                    nc.tensor.matmul(
                                        psum[:, li * 128:(li + 1) * 128], lhsT,
                                        KV[:, mi * 128:(mi + 1) * 128],
                                        start=(j == 0 and li % 4 == 0),
                                        stop=(j == li and li % 4 == 3))
                        return psum

                    def emit_back(h, psum):
                        KS, KV, X1R = front.pop(h)
                        g = gp.tile([128, NT * 128], f16, tag="g")
                        if direct_gate:
                            for bank in range(NT // 4 if gate else 0):
                                sl = slice(bank * 512, bank * 512 + 512)
                                nc.vector.tensor_tensor(out=g[:, sl], in0=psum[:, sl], in1=X1R[:, sl], op=MUL)
                        else:
                            y16 = y16p.tile([128, NT * 128], f16, tag="y16")
                            for bank in range(NT // 4 if gate else 0):
                                sl = slice(bank * 512, bank * 512 + 512)
                                nc.scalar.copy(out=y16[:, sl], in_=psum[:, sl])
                                nc.vector.tensor_tensor(out=g[:, sl], in0=y16[:, sl], in1=X1R[:, sl], op=MUL)
                        for b in range(B):
                            for width in (4, 2, 1):
                                a0 = bass.AP(g[:].tensor, g[:].offset + b * NH * NH,
                                             [[NT * 128, 128], [128, NT], [NH, width], [1, NH]])
                                a1 = bass.AP(g[:].tensor, g[:].offset + b * NH * NH + width * NH,
                                             [[NT * 128, 128], [128, NT], [NH, width], [1, NH]])
                                nc.vector.tensor_tensor(out=a0, in0=a0, in1=a1, op=ADD)
                        o32 = bass.AP(outT32[:].tensor, outT32[:].offset + h * NH,
                                      [[B * NT * DL, 128], [NT * DL, B], [DL, NT], [1, NH]])
                        i_x2 = bass.AP(x2T[:].tensor, x2T[:].offset + h * NH,
                                       [[B * NT * DL, 128], [NT * DL, B], [DL, NT], [1, NH]])
                        i_wt = bass.AP(WT[:].tensor, WT[:].offset + h,
                                       [[B * NT * HL, 128], [NT * HL, B], [HL, NT], [0, NH]])
                        nc.vector.tensor_tensor(out=o32, in0=i_x2, in1=i_wt, op=MUL)
                        i_g = bass.AP(g[:].tensor, g[:].offset,
                                      [[NT * 128, 128], [NH * NH, B], [128, NT], [1, NH]])
                        nc.vector.tensor_tensor(out=o32, in0=o32, in1=i_g, op=ADD)

                    # software pipeline: next head's KS/KV/X1R are emitted
                    # before this head's gate stage so DVE produces KV(h+1)
                    # ahead of its own gate tail and PE never waits
                    if heads > 0:
                        emit_front(0)
                    for h in range(heads):
                        psum = emit_conv(h)
                        if h + 1 < heads:
                            emit_front(h + 1)
                        emit_back(h, psum)

            # ---- transpose back to natural + store ----
            with tc.tile_pool(name="pso", bufs=4, space="PSUM") as pso:
                for b in range(B):
                    for lc in range(NT):
                        ps = pso.tile([DL, 128], f32, tag="pso")
                        col = (b * NT + lc) * DL
                        nc.tensor.transpose(ps[:], outT32[:, col:col + DL], ids[:])
                        nc.scalar.copy(out=out_nat[:, b * L + lc * 128: b * L + (lc + 1) * 128],
                                       in_=ps[:])
                for b in range(B):
                    nc.sync.dma_start(out=out_d[b], in_=out_nat[:, b * L:(b + 1) * L])
    return nc


def _build2(nc: bass.Bass, heads: int = HL, conv: bool = True, gate: bool = True,
            reps: int = 1, kv_eng: str = "pool", gate_mode: str = "direct",
            final_eng: str = "pool", epilogue: str = "dma_inc",
            ks_prefetch: bool = True, back: int = 3, ks_dma: bool = True,
            same_ks: bool = False, kv_on: bool = True, conv_mode: str = "bank",
            wrap_all: bool = False, depth: int = 1, tail_dve: int = 0,
            ks_group: int = 1, ring_split: bool = True, kv0_cross: bool = False):
    """Redesign of _build: ACT leaves the per-head loop entirely (no X1R
    replication, no psum drain), KV formation moves to the otherwise-idle
    Pool engine, the gate reads PSUM directly with a d2-broadcast x1 AP,
    and the output is stored head-major so each head's result DMAs out
    incrementally (host detangles the layout).
    Engine budget per head: PE 7.3us conv / DVE gate+reduce ~3us /
    Pool KV+final ~2.2us / ACT idle."""
    # inputs arrive HOST-PRE-TRANSPOSED in [128 lam, (b, lc, d)] layout:
    # plain 1MB contiguous DMAs instead of 128 xbar transpose descriptors
    # (measured ~10GB/s effective for dma_start_transpose — dominated the
    # prologue at ~70us; plain DMAs take ~6us/ring).
    vT_d = nc.dram_tensor("vT16", [128, B * NT * DL], f16, kind="ExternalInput")
    x1T_d = nc.dram_tensor("x1T16", [128, B * NT * DL], f16, kind="ExternalInput")
    x1bT_d = nc.dram_tensor("x1bT16", [128, B * NT * DL], f16, kind="ExternalInput")
    x2T_d = nc.dram_tensor("x2T16", [128, B * NT * DL], f16, kind="ExternalInput")
    kstag16 = nc.dram_tensor("kstag16", [HL, 128, KP], f16, kind="ExternalInput")
    if epilogue == "pe":
        ident_s = nc.dram_tensor("ident_s", [128, 128], f32, kind="ExternalInput")
        out_d = nc.dram_tensor("out", [B, DL, L], f32, kind="ExternalOutput")
    else:
        out_d = nc.dram_tensor("out", [128, HL * B * NT * NH], f32, kind="ExternalOutput")

    kv_e = nc.gpsimd if kv_eng == "pool" else nc.vector
    fin_e = nc.gpsimd if final_eng == "pool" else nc.vector

    with TileContext(nc) as tc:
        with tc.tile_pool(name="const", bufs=1) as constp, \
             tc.tile_pool(name="persist", bufs=1) as persist:
            if epilogue == "pe":
                ids = constp.tile([128, 128], f32)
                nc.sync.dma_start(out=ids[:], in_=ident_s[:])

            # persistent transposed inputs: [128 lam, (b, lc, d)] fp16.
            # Ping-ponged (nsets=2) in wrap_all timing builds so iteration
            # i+1's input DMAs overlap iteration i's compute.
            nsets = 2 if (wrap_all and reps > 1) else 1
            vTs, x1Ts, x1bTs, x2Ts, WTs = [], [], [], [], []
            for s in range(nsets):
                vT_ = persist.tile([128, B * NT * DL], f16, name=f"vT{s}")
                x1T_ = persist.tile([128, B * NT * DL], f16, name=f"x1T{s}")
                x1bT_ = persist.tile([128, B * NT * DL], f16, name=f"x1bT{s}")
                x2T_ = persist.tile([128, B * NT * DL], f16, name=f"x2T{s}")
                WT_ = persist.tile([128, B * NT * HL], f32, name=f"WT{s}")
                vTs.append(vT_); x1Ts.append(x1T_); x1bTs.append(x1bT_)
                x2Ts.append(x2T_); WTs.append(WT_)
            # head-major output: col = h*(B*NT*NH) + b*(NT*NH) + lc*NH + d2
            outT32 = persist.tile([128, HL * B * NT * NH], f32)
            if epilogue == "pe":
                out_nat = persist.tile([DL, B * L], f32)

            def emit_prologue(par):
                vT, x1T, x1bT, x2T, WT = (vTs[par], x1Ts[par], x1bTs[par],
                                          x2Ts[par], WTs[par])
                # ---- input loads (host pre-transposed, plain DMAs) ----
                # ALL loads go on the sync ring; the scalar ring carries only
                # output stores.  Rings are in-order queues: if a next-body
                # load sits behind an out-DMA gated on this body's final
                # head, the load serializes at the body boundary (~5-8us).
                # With the split, every load's WAR sem fires a full body
                # ahead of when its data is needed.
                x1_ring = nc.sync if ring_split else nc.scalar
                nc.sync.dma_start(out=vT[:], in_=vT_d[:])
                nc.sync.dma_start(out=x2T[:], in_=x2T_d[:])
                x1_ring.dma_start(out=x1T[:], in_=x1T_d[:])
                x1_ring.dma_start(out=x1bT[:], in_=x1bT_d[:])

                # ---- W term: WT[lam, (b, lc, h)] = sum_d1 bias*x1*v ----
                # (2 ops: fused product + single innermost-axis reduce)
                t = wtmp.tile([128, B * NT * DL], f16, tag=f"wt{par}")
                nc.vector.tensor_tensor(out=t[:], in0=x1bT[:], in1=vT[:], op=MUL)
                tin = bass.AP(t[:].tensor, t[:].offset,
                              [[B * NT * DL, 128], [DL, B * NT], [NH, HL], [1, NH]])
                nc.vector.tensor_reduce(out=WT[:], in_=tin,
                                        axis=mybir.AxisListType.X, op=ADD)

            # ---- main per-head loop ----
            import contextlib
            with tc.tile_pool(name="wtmp", bufs=1) as wtmp, \
                 tc.tile_pool(name="ks", bufs=HL) as ksp, \
                 tc.tile_pool(name="kv", bufs=3) as kvp, \
                 tc.tile_pool(name="y16", bufs=3) as y16p, \
                 tc.tile_pool(name="g", bufs=3) as gp, \
                 tc.tile_pool(name="psc", bufs=2, space="PSUM") as psc:
                pending_kv0 = {}

                def emit_kv0(par):
                    # cross-body prefetch: next body's KV(0) emitted ahead of
                    # this body's tail finals so Pool has it ready and the PE
                    # crosses the body boundary without waiting
                    vT_n, x2T_n = vTs[par], x2Ts[par]
                    KV = kvp.tile([128, NT * 128], f16, tag="kv")
                    for b in range(B):
                        i0 = bass.AP(vT_n[:].tensor, vT_n[:].offset + b * NT * DL,
                                     [[B * NT * DL, 128], [DL, NT], [1, NH], [0, NH]])
                        i1 = bass.AP(x2T_n[:].tensor, x2T_n[:].offset + b * NT * DL,
                                     [[B * NT * DL, 128], [DL, NT], [0, NH], [1, NH]])
                        o = bass.AP(KV[:].tensor, KV[:].offset + b * NH * NH,
                                    [[NT * 128, 128], [128, NT], [NH, NH], [1, NH]])
                        kv_e.tensor_tensor(out=o, in0=i0, in1=i1, op=MUL)
                    pending_kv0[par] = KV

                def emit_body(par, kv0_next=None):
                    vT, x1T, x1bT, x2T, WT = (vTs[par], x1Ts[par], x1bTs[par],
                                              x2Ts[par], WTs[par])
                    front = {}
                    KS_all = {}
                    if ks_prefetch and ks_group > 1:
                        # grouped KS prefetch: one 3-dim diagonal-AP DMA per
                        # ks_group heads -> heads in a group share a single
                        # DMA-completion sem (fewer PE wait points)
                        assert heads % ks_group == 0
                        for g0 in range(0, heads, ks_group):
                            KSg = ksp.tile([128, ks_group * L], f16, tag="ksg",
                                           bufs=heads // ks_group)
                            ksrc = bass.AP(kstag16[:].tensor, g0 * 128 * KP + 127,
                                           [[KP - 1, 128], [128 * KP, ks_group], [1, L]])
                            eng = nc.sync if (g0 // ks_group) % 2 == 0 else nc.scalar
                            eng.dma_start(out=KSg[:], in_=ksrc)
                            for hh in range(ks_group):
                                KS_all[g0 + hh] = (KSg, hh * L)
                    elif ks_prefetch:
                        for h in range(heads):
                            KS = ksp.tile([128, L], f16, tag="ks")
                            if ks_dma:
                                ksrc = bass.AP(kstag16[:].tensor, h * 128 * KP + 127,
                                               [[KP - 1, 128], [1, L]])
                                eng = nc.sync if (ring_split or h % 2 == 0) else nc.scalar
                                eng.dma_start(out=KS[:], in_=ksrc)
                            else:  # timing ablation: token write only
                                ksrc = bass.AP(kstag16[:].tensor, h * 128 * KP + 127,
                                               [[KP - 1, 128], [1, 1]])
                                nc.sync.dma_start(out=KS[:, 0:1], in_=ksrc)
                            KS_all[h] = (KS, 0)

                    def emit_front(h):
                        if ks_prefetch:
                            KS = KS_all[h]
                        else:
                            KSt = ksp.tile([128, L], f16, tag="ks")
                            ksrc = bass.AP(kstag16[:].tensor, h * 128 * KP + 127,
                                           [[KP - 1, 128], [1, L]])
                            nc.sync.dma_start(out=KSt[:], in_=ksrc)
                            KS = (KSt, 0)
                        if h == 0 and pending_kv0.get(par) is not None:
                            front[0] = (KS, pending_kv0.pop(par))
                            return
                        # KV[l, lc*128 + b*64 + d1*8 + d2] = v*x2
                        # (ISA TENSOR3D: max 3 free dims -> one op per batch)
                        KV = kvp.tile([128, NT * 128], f16, tag="kv")
                        for b in range(B):
                            nd = [NT, NH, NH] if kv_on else [1, 1, 1]
                            i0 = bass.AP(vT[:].tensor, vT[:].offset + b * NT * DL + h * NH,
                                         [[B * NT * DL, 128], [DL, nd[0]], [1, nd[1]], [0, nd[2]]])
                            i1 = bass.AP(x2T[:].tensor, x2T[:].offset + b * NT * DL + h * NH,
                                         [[B * NT * DL, 128], [DL, nd[0]], [0, nd[1]], [1, nd[2]]])
                            o = bass.AP(KV[:].tensor, KV[:].offset + b * NH * NH,
                                        [[NT * 128, 128], [128, nd[0]], [NH, nd[1]], [1, nd[2]]])
                            kv_e.tensor_tensor(out=o, in0=i0, in1=i1, op=MUL)
                        front[h] = (KS, KV)

                    def emit_conv(h):
                        (KSt, ko), KV = front[h]
                        psum = psc.tile([128, NT * 128], f32, tag="ps")
                        NJ = NT if conv is True else int(conv)
                        for j in range(NJ):
                            js = 0 if same_ks else j
                            lhsT = KSt[:, ko + js * 128:ko + (js + 1) * 128]
                            if conv_mode == "span":
                                # one full-span matmul per diagonal: fewer PE
                                # instructions; stop is sim-only bookkeeping
                                nc.tensor.matmul(
                                    psum[:, j * 128:NT * 128], lhsT,
                                    KV[:, 0:(NT - j) * 128],
                                    start=(j == 0), stop=(j == NJ - 1),
                                    skip_group_check=True)
                                continue
                            for bk in range(NT // 4):
                                lo = max(j, 4 * bk)
                                hi = 4 * bk + 3
                                if lo > hi:
                                    continue
                                cnt = hi - lo + 1
                                mi0 = lo - j
                                nc.tensor.matmul(
                                    psum[:, lo * 128:(hi + 1) * 128], lhsT,
                                    KV[:, mi0 * 128:(mi0 + cnt) * 128],
                                    start=(j == 0), stop=(j == min(hi, NJ - 1)))
                        return psum

                    def emit_back(h, psum):
                        KS, KV = front.pop(h)
                        g = gp.tile([128, NT * 128], f16, tag="g")
                        if gate_mode != "direct":
                            y16 = y16p.tile([128, NT * 128], f16, tag="y16")
                        if gate and back >= 1:
                            for bk in range(NT // 4):
                                if gate_mode != "direct":
                                    sl = slice(bk * 512, bk * 512 + 512)
                                    nc.scalar.copy(out=y16[:, sl], in_=psum[:, sl])
                                for b in range(B):
                                    # per-bank gate straight from PSUM with
                                    # d2-broadcast x1 (no X1R, no ACT drain)
                                    c0 = bk * 512 + b * NH * NH
                                    o = bass.AP(g[:].tensor, g[:].offset + c0,
                                                [[NT * 128, 128], [128, 4], [NH, NH], [1, NH]])
                                    ix = bass.AP(x1T[:].tensor,
                                                 x1T[:].offset + (b * NT + bk * 4) * DL + h * NH,
                                                 [[B * NT * DL, 128], [DL, 4], [1, NH], [0, NH]])
                                    if gate_mode == "direct":
                                        ip = bass.AP(psum[:].tensor, psum[:].offset + c0,
                                                     [[NT * 128, 128], [128, 4], [NH, NH], [1, NH]])
                                    else:  # drain16: gate from fp16 drain
                                        ip = bass.AP(y16[:].tensor, y16[:].offset + c0,
                                                     [[NT * 128, 128], [128, 4], [NH, NH], [1, NH]])
                                    nc.vector.tensor_tensor(out=o, in0=ip, in1=ix, op=MUL)
                        # d1-reduction tree (in place on g; result in d1=0 cols)
                        for b in range(B if back >= 2 else 0):
                            for width in (4, 2, 1):
                                a0 = bass.AP(g[:].tensor, g[:].offset + b * NH * NH,
                                             [[NT * 128, 128], [128, NT], [NH, width], [1, NH]])
                                a1 = bass.AP(g[:].tensor, g[:].offset + b * NH * NH + width * NH,
                                             [[NT * 128, 128], [128, NT], [NH, width], [1, NH]])
                                nc.vector.tensor_tensor(out=a0, in0=a0, in1=a1, op=ADD)
                        # final: outT32[h block] = x2*W + reduced gate (Pool)
                        if back < 3:
                            return
                        if epilogue == "pe":
                            # natural (b, lc, d)-major layout
                            o32 = bass.AP(outT32[:].tensor, outT32[:].offset + h * NH,
                                          [[B * NT * DL, 128], [NT * DL, B], [DL, NT], [1, NH]])
                        else:
                            o32 = bass.AP(outT32[:].tensor, outT32[:].offset + h * B * NT * NH,
                                          [[HL * B * NT * NH, 128], [NT * NH, B], [NH, NT], [1, NH]])
                        i_x2 = bass.AP(x2T[:].tensor, x2T[:].offset + h * NH,
                                       [[B * NT * DL, 128], [NT * DL, B], [DL, NT], [1, NH]])
                        i_wt = bass.AP(WT[:].tensor, WT[:].offset + h,
                                       [[B * NT * HL, 128], [NT * HL, B], [HL, NT], [0, NH]])
                        # route the last heads' final ops off Pool so the next
                        # body's KV(0) isn't queued behind them at the boundary
                        fe = nc.vector if h >= heads - tail_dve else fin_e
                        fe.tensor_tensor(out=o32, in0=i_x2, in1=i_wt, op=MUL)
                        i_g = bass.AP(g[:].tensor, g[:].offset,
                                      [[NT * 128, 128], [NH * NH, B], [128, NT], [1, NH]])
                        fe.tensor_tensor(out=o32, in0=o32, in1=i_g, op=ADD)
                        if epilogue == "dma_inc" and (reps == 1 or wrap_all):
                            c0 = h * B * NT * NH
                            eng = (nc.scalar if ring_split
                                   else (nc.sync if h % 2 == 0 else nc.scalar))
                            eng.dma_start(
                                out=bass.AP(out_d[:].tensor, c0,
                                            [[HL * B * NT * NH, 128], [1, B * NT * NH]]),
                                in_=outT32[:, c0:c0 + B * NT * NH])

                    for h0 in range(min(depth, heads)):
                        emit_front(h0)
                    for h in range(heads):
                        psum = emit_conv(h)
                        if h + depth < heads:
                            emit_front(h + depth)
                        if h == heads - 2 and kv0_next is not None:
                            emit_kv0(kv0_next)
                        emit_back(h, psum)

                if wrap_all and reps > 1:
                    with tc.For_i(0, (reps - 1) // nsets, 1):
                        for par in range(nsets):
                            emit_prologue(par)
                            emit_body(par, kv0_next=((par + 1) % nsets
                                      if kv0_cross else None))
                else:
                    emit_prologue(0)
                    rep_ctx = (tc.For_i(0, reps, 1) if reps > 1
                               else contextlib.nullcontext())
                    with rep_ctx:
                        emit_body(0)

            # ---- epilogue ----
            if epilogue == "pe":
                with tc.tile_pool(name="pso", bufs=4, space="PSUM") as pso:
                    for b in range(B):
                        for lc in range(NT):
                            ps = pso.tile([DL, 128], f32, tag="pso")
                            src = bass.AP(outT32[:].tensor,
                                          outT32[:].offset + (b * NT + lc) * DL,
                                          [[B * NT * DL, 128], [1, DL]])
                            nc.tensor.transpose(ps[:], src, ids[:])
                            nc.scalar.copy(out=out_nat[:, b * L + lc * 128: b * L + (lc + 1) * 128],
                                           in_=ps[:])
                    for b in range(B):
                        nc.sync.dma_start(out=out_d[b], in_=out_nat[:, b * L:(b + 1) * L])
            elif reps != 1 and back >= 3 and not wrap_all:  # timing: bulk store
                nc.sync.dma_start(out=out_d[:], in_=outT32[:])
            elif back < 3:  # ablation builds never write outT32: token store
                nc.sync.dma_start(out=out_d[:, 0:1], in_=WTs[0][:, 0:1])
    return nc


def _probe(nc: bass.Bass, heads: int = HL, reps: int = 1, conv: bool = True,
           n_ks: int = HL, psum_bufs: int = 1):
    """Timing probe: the pure conv matmul stream with ZERO semaphore waits
    inside the loop (KS/KV static, single PSUM tile recycled in-order).
    Measures the PE stream ceiling absent cross-engine sync."""
    kstag16 = nc.dram_tensor("kstag16", [HL, 128, KP], f16, kind="ExternalInput")
    out_d = nc.dram_tensor("out", [128, 128], f16, kind="ExternalOutput")
    with TileContext(nc) as tc:
        with tc.tile_pool(name="static", bufs=1) as statp, \
             tc.tile_pool(name="psc", bufs=psum_bufs, space="PSUM") as psc:
            KS_all = {}
            for hh in range(n_ks):
                KS = statp.tile([128, L], f16, tag=f"ks{hh}")
                ksrc = bass.AP(kstag16[:].tensor, hh * 128 * KP + 127,
                               [[KP - 1, 128], [1, L]])
                eng = nc.sync if hh % 2 == 0 else nc.scalar
                eng.dma_start(out=KS[:], in_=ksrc)
                KS_all[hh] = KS
            KV = statp.tile([128, NT * 128], f16, tag="kv")
            nc.vector.tensor_tensor(out=KV[:], in0=KS_all[0][:], in1=KS_all[n_ks - 1][:], op=MUL)
            import contextlib
            rep_ctx = tc.For_i(0, reps, 1) if reps > 1 else contextlib.nullcontext()
            with rep_ctx:
                for h in range(heads):
                    KS = KS_all[h % n_ks]
                    psum = psc.tile([128, NT * 128], f32, tag="ps")
                    NJ = NT if conv is True else int(conv)
                    for j in range(NJ):
                        lhsT = KS[:, j * 128:(j + 1) * 128]
                        for bk in range(NT // 4):
                            lo = max(j, 4 * bk)
                            hi = 4 * bk + 3
                            if lo > hi:
                                continue
                            nc.tensor.matmul(
                                psum[:, lo * 128:(hi + 1) * 128], lhsT,
                                KV[:, (lo - j) * 128:(hi - j + 1) * 128],
                                start=(j == 0), stop=(j == min(hi, NJ - 1)),
                                skip_group_check=True)
            nc.sync.dma_start(out=out_d[:], in_=KV[:, 0:128])
    return nc


_NC_CACHE = {}

BUILD = _build2
BUILD_KW = {}


def _get_nc():
    key = ("nc", BUILD.__name__, tuple(sorted(BUILD_KW.items())))
    if key not in _NC_CACHE:
        nc = bass.Bass()
        BUILD(nc, **BUILD_KW)
        _NC_CACHE[key] = nc
    return _NC_CACHE[key]


def _to_lam(a16):
    """[B, DL, L] -> [128 lam, (b, lc, d)] host pre-transpose."""
    return np.ascontiguousarray(
        a16.reshape(B, DL, NT, 128).transpose(3, 0, 2, 1).reshape(128, B * NT * DL))


def make_in_maps(v, k, bias, x1, x2):
    v16 = np.asarray(v, np.float32).astype(np.float16)
    x1_16 = np.asarray(x1, np.float32).astype(np.float16)
    x2_16 = np.asarray(x2, np.float32).astype(np.float16)
    k32 = np.asarray(k, np.float32)
    bias16 = np.asarray(bias, np.float32).astype(np.float16)
    x1b16 = (np.asarray(x1, np.float32)
             * np.asarray(bias, np.float32)[None, :, None]).astype(np.float16)
    kpad = np.zeros((H, KP), np.float16)
    kpad[:, 127:] = k32.astype(np.float16)
    ident_h = np.eye(DL, dtype=np.float16)
    ident_s = np.eye(128, dtype=np.float32)
    in_maps = []
    for c in range(N_CORES):
        dsl = slice(c * DL, (c + 1) * DL)
        hsl = slice(c * HL, (c + 1) * HL)
        in_maps.append({
            # pre-transposed inputs for _build2
            "vT16": _to_lam(v16[:, dsl, :]),
            "x1T16": _to_lam(x1_16[:, dsl, :]),
            "x1bT16": _to_lam(x1b16[:, dsl, :]),
            "x2T16": _to_lam(x2_16[:, dsl, :]),
            # natural-layout inputs for the legacy _build path
            "v16": np.ascontiguousarray(v16[:, dsl, :]),
            "x1_16": np.ascontiguousarray(x1_16[:, dsl, :]),
            "x1b16": np.ascontiguousarray(x1b16[:, dsl, :]),
            "x2_16": np.ascontiguousarray(x2_16[:, dsl, :]),
            "kstag16": np.ascontiguousarray(np.broadcast_to(
                kpad[hsl][:, None, :], (HL, 128, KP))),
            "ident_h": ident_h,
            "identb_h": np.diag(bias16[dsl]).astype(np.float16),
            "ident_s": ident_s,
        })
    return in_maps


def kernel(v, k, bias, x1, x2, num_heads):
    assert int(num_heads) == NH
    in_maps = make_in_maps(v, k, bias, x1, x2)
    res = run_bass_kernel_spmd(_get_nc(), in_maps, list(range(N_CORES)))
    chunks = []
    for c in range(N_CORES):
        arr = res.results[c]["out"]
        if arr.shape == (B, DL, L):  # natural-layout epilogue
            chunks.append(arr)
        else:  # head-major transposed layout: arr[lam, (h, b, lc, d2)]
            chunks.append(np.ascontiguousarray(
                arr.reshape(128, HL, B, NT, NH).transpose(2, 1, 4, 3, 0)
            ).reshape(B, DL, L))
    return np.concatenate(chunks, axis=1).astype(np.float32)

